# revision 17
# baseline (speedup 1.0000x reference)
"""AreaAttentionBlock Trainium2 kernel (8 NeuronCores, data-parallel).

Problem: B=2, C=256, H=W=64, HEADS=8 (hd=32), AREA=4, MLP_DIM=307.
One (batch, area) group of 1024 pixels per core; the only cross-slab
dependency is the 1-row halo of the depthwise 3x3 conv, host-supplied.

Per-core pipeline:
  - All 1x1 convs run as fp8e4 DoubleRow matmuls (two K=128 k-tiles ride
    the DR pair dim). Weights host-scaled x64 into fp8's normal range;
    1/64 rides the psum->sbuf tensor_scalar ops; conv biases folded into
    host-prepared xf tiles.
  - Depthwise 3x3 conv on PE as diagonal-matrix fp8 DR matmuls over the
    zero-padded 18x66 v4 layout (tap pairs via overlapping strided APs).
  - Attention in 16 sub-blocks (ncc, hg, half): scores bf16 K=32;
    exp on ACT writes fp8 directly into j-pair plane tiles (pt2);
    AV+colsum fused: one fp8 DR M=128 matmul per (j-pair, head) with
    zero-padded window lhsT [vT_h|ones|0|0] so both heads of a half
    accumulate into ONE [128,512] psum tile holding [av|cs|av'|cs']
    (window order w=4hg+2m+half keeps vt scatter APs 3-free-dim).
    Norm: copy psum->sbuf, recip, 8 partition-shift gather DMAs build
    compact av and 1/cs, one mul; head order [0,2,1,3] absorbed into
    the host proj-weight permutation.
  - MLP silu via tanh (stays in exp ACT table set).
The HAM power governor throttles PE to 50% duty when PE streams exceed
~50% of wall; the DR design keeps attention-phase PE at ~48% so the exp
phase stays ACT-bound even when throttled.
"""

import numpy as np
import ml_dtypes

C = 256
HEADS = 8
HD = 32
AREA = 4
MLP = 307
B, H, W = 2, 64, 64
NPX = 1024          # pixels per slab (16 rows)
NHALO = 1152        # 18 rows with halo
SCALE = float(1.0 / np.sqrt(HD))
WS = 64.0           # host weight scale into fp8 normal range
IWS = float(1.0 / WS)

BF16 = ml_dtypes.bfloat16
F8 = ml_dtypes.float8_e4m3

# w1 free layout per k-tile: [qT 256 | kT 256 | vext 512 | vdense 256];
# vext has [v-cols(32) | zero-cols(32)] per head in window order (the
# zero block becomes the ones block via the bias matmul writing WS);
# vdense is the v weights densely in attn2's permuted head order, used
# by the v4 (image-layout) conv so pe matches attn2's channel order.
W1KT = 1280

# w2 packing offsets (wproj | wm1 | wm2 along free dim)
W2_PROJ = 0            # 2 x 256
W2_M1 = 512            # 2 x 384 (307 zero-padded to 16-aligned DR stride)
W2_M2 = 512 + 768      # 3 x 256
W2_TOT = W2_M2 + 768

# ball (f32 [128, 34]) column map
BQ, BK, BV = 0, 2, 4
BM1, BM1H = 10, 13

# dwdiag packing: per g, 4 DR pair tiles [128,2,128] + 1 single [128,128]
DW_PAIRS = [(65, 131), (66, 132), (67, 133), (197, 199)]
DW_SINGLE = 198
DW_TAP_OF_OFF = {65: 0, 66: 1, 67: 2, 131: 3, 132: 4, 133: 5,
                 197: 6, 198: 7, 199: 8}
DWG = 4 * 256 + 128    # 1152 cols per g

_COMPILED = {}


def _build_graph():
    import concourse.bacc as bacc
    import concourse.mybir as mybir
    import concourse.tile as tile
    from concourse.tile import add_dep_helper

    f32 = mybir.dt.float32
    bf16 = mybir.dt.bfloat16
    f8 = mybir.dt.float8e4
    DR = mybir.MatmulPerfMode.DoubleRow
    AF = mybir.ActivationFunctionType
    OP = mybir.AluOpType

    nc = bacc.Bacc(target_bir_lowering=False)

    xf1_d = nc.dram_tensor("xf1", [2, 128, NPX], f32, kind="ExternalInput")
    xf2_d = nc.dram_tensor("xf2", [2, 128, NPX], f32, kind="ExternalInput")
    xb_d = nc.dram_tensor("xb", [128, 2, NHALO], f8, kind="ExternalInput")
    w1_d = nc.dram_tensor("w1", [128, 2 * W1KT], f8, kind="ExternalInput")
    w2_d = nc.dram_tensor("w2", [128, W2_TOT], f8, kind="ExternalInput")
    dw_d = nc.dram_tensor("dw", [128, 2 * DWG], f8, kind="ExternalInput")
    ball_d = nc.dram_tensor("ball", [128, 34], f32, kind="ExternalInput")
    bvrow_d = nc.dram_tensor("bvrow", [1, 512], bf16, kind="ExternalInput")
    out_d = nc.dram_tensor("out", [2, 128, NPX], f32, kind="ExternalOutput")

    with tile.TileContext(nc) as tc:
        with (
            tc.sbuf_pool(name="weights", bufs=1) as wp,
            tc.sbuf_pool(name="acts", bufs=1) as ap,
            tc.sbuf_pool(name="pt_pool", bufs=3) as ptp,
            tc.sbuf_pool(name="small", bufs=2) as sp,
            tc.psum_pool(name="ps", bufs=1) as psp,
        ):
            # constants / ACT table preload
            onesrow = wp.tile([1, 128], bf16, name="onesrow")
            nc.vector.memset(onesrow[:], 1.0)
            warm = wp.tile([1, 16], f32, name="warm")
            # loads the exp ACT table set during the DMA phase
            nc.scalar.activation(warm[:], onesrow[:, 0:16], AF.Exp)

            # DMAs (ordered by first use)
            xb = ap.tile([128, 2 * NHALO], f8, name="xb")
            w1 = wp.tile([128, 2 * W1KT], f8, name="w1")
            ball = wp.tile([128, 34], f32, name="ball")

            def xbv():
                return xb[:].rearrange("p (k n) -> p k n", k=2)

            def w1v():
                return w1[:].rearrange("p (k m) -> p k m", k=2)

            nc.sync.dma_start(out=xbv()[:, :, 0:576],
                              in_=xb_d[:, :, 0:576])
            nc.gpsimd.dma_start(
                out=w1v()[:, :, 0:512],
                in_=w1_d[:].rearrange("p (k m) -> p k m", k=2)[:, :, 0:512])
            nc.sync.dma_start(out=ball[:], in_=ball_d[:])
            nc.gpsimd.dma_start(out=xbv()[:, :, 576:1152],
                                in_=xb_d[:, :, 576:1152])
            nc.sync.dma_start(
                out=w1v()[:, :, 512:1280],
                in_=w1_d[:].rearrange("p (k m) -> p k m", k=2)[:, :, 512:1280])
            bvrow = wp.tile([1, 512], bf16, name="bvrow")
            nc.gpsimd.dma_start(out=bvrow[:], in_=bvrow_d[:])
            w2 = wp.tile([128, W2_TOT], f8, name="w2")
            nc.gpsimd.dma_start(out=w2[:], in_=w2_d[:])
            dwdiag = wp.tile([128, 2 * DWG], f8, name="dwdiag")
            nc.gpsimd.dma_start(out=dwdiag[:], in_=dw_d[:])
            xf1 = [ap.tile([128, NPX], f32, name=f"xf1{k}") for k in range(2)]
            xf2 = [ap.tile([128, NPX], f32, name=f"xf2{k}") for k in range(2)]
            for k in range(2):
                nc.gpsimd.dma_start(out=xf1[k][:], in_=xf1_d[k])
                nc.sync.dma_start(out=xf2[k][:], in_=xf2_d[k])

            # persistent activation tiles
            q_sb = [ap.tile([128, NPX], bf16, name=f"q{g}") for g in range(2)]
            k_sb = [ap.tile([128, NPX], bf16, name=f"k{g}") for g in range(2)]
            # vT2[jp]: [pl(2) x 8 windows x 128] fp8; window w=4hg+2m+half:
            #   half0 m0: [vT|1|0|0]   half0 m1: [0|0|vT|1]
            #   half1 m0: [1|vT|0|0]   half1 m1: [0|0|1|vT]
            vT2 = [ap.tile([128, 2048], f8, name=f"vT2{jp}")
                   for jp in range(4)]
            for jp in range(4):
                nc.vector.memset(vT2[jp][:], 0.0)
            v4pad = [ap.tile([128, 1256], f8, name=f"v4p{g}")
                     for g in range(2)]
            pe_sb = [ap.tile([128, 1056], bf16, name=f"pe{g}")
                     for g in range(2)]
            attn2 = ap.tile([128, 2 * NPX], f8, name="attn2")
            x1f = [ap.tile([128, NPX], f32, name=f"x1f{g}") for g in range(2)]
            x1b2 = ap.tile([128, 2 * NPX], f8, name="x1b2")
            u01 = ap.tile([128, 2 * NPX], f8, name="u01")
            u2 = ap.tile([128, NPX], f8, name="u2")
            out_sb = [ap.tile([128, NPX], f32, name=f"osb{g}")
                      for g in range(2)]

            for g in range(2):
                nc.vector.memset(v4pad[g][:], 0.0)

            v4_insts = {0: [], 1: []}
            dw_last = {}

            # ---- conv building blocks ----
            def qk_conv_chunk(which, g, ncc):
                """One 512-px chunk of the q or k 1x1 conv (fp8 DR)."""
                dst = (q_sb, k_sb)[which]
                bias_col = (BQ, BK)[which] + g
                ps = psp.tile([128, 512], f32, tag="acc", name="qkc", bufs=2)
                mt = 256 * which + 128 * g
                nc.tensor.matmul(
                    ps[:],
                    lhsT=w1v()[:, :, mt : mt + 128],
                    rhs=xbv()[:, :, 64 + 512 * ncc : 64 + 512 * ncc + 512],
                    start=True, stop=True,
                    perf_mode=DR,
                    skip_group_check=True,
                )
                nc.vector.tensor_scalar(
                    out=dst[g][:, 512 * ncc : 512 * ncc + 512], in0=ps[:],
                    scalar1=IWS, scalar2=ball[:, bias_col : bias_col + 1],
                    op0=OP.mult, op1=OP.add,
                )

            def _win_ap(t, base, dims):
                """AP at column `base` with free dims `dims` ([stride,size]
                pairs) plus the trailing [1,32] block."""
                a = t[:, base : base + 32]
                for _ in dims:
                    a = a.unsqueeze(1)
                for i, ss in enumerate(dims):
                    a.ap[1 + i] = list(ss)
                return a

            def vt_conv(p):
                """V^T px-tile p -> vT2[p//2] plane p%2 window scatter."""
                ps = psp.tile([128, 512], f32, tag="acc", name="vtc", bufs=2)
                px0 = 64 + 128 * p
                nc.tensor.matmul(
                    ps[:],
                    lhsT=xbv()[:, :, px0 : px0 + 128],
                    rhs=w1v()[:, :, 512:1024],
                    start=True, stop=False,
                    perf_mode=DR,
                    skip_group_check=True,
                )
                nc.tensor.matmul(
                    ps[:], lhsT=onesrow[:], rhs=bvrow[:],
                    start=False, stop=True, skip_group_check=True,
                )
                jp, pl = p // 2, p % 2
                t = vT2[jp]
                base = 1024 * pl
                # head h=4hg+2half+m; psum v-col (host vext order):
                # 256hg+128m+64half; window w=4hg+2m+half at 128w with
                # inwin v at 64m+32half -> out = 512hg+320m+160half.
                # Both nest uniformly over k=2m+half (one 3-dim op).
                ov = _win_ap(t, base, [(512, 2), (160, 4)])
                iv = _win_ap(ps, 0, [(256, 2), (64, 4)])
                nc.vector.tensor_scalar_mul(out=ov, in0=iv, scalar1=IWS)
                # ones: out 512hg+320m+96half+32, in 256hg+128m+64half+32
                # -- split per hg to stay 3-dim
                for hg in range(2):
                    oo = _win_ap(t, base + 512 * hg + 32,
                                 [(320, 2), (96, 2)])
                    io = _win_ap(ps, 32 + 256 * hg, [(128, 2), (64, 2)])
                    nc.vector.tensor_scalar_mul(out=oo, in0=io, scalar1=IWS)

            def v4_chunk(g, c0, cw):
                """One chunk of the v 1x1 conv into the padded 18x66 layout."""
                ps = psp.tile([128, 512], f32, tag="acc", name="v4c", bufs=2)
                nc.tensor.matmul(
                    ps[:, 0:cw],
                    lhsT=w1v()[:, :, 1024 + 128 * g : 1152 + 128 * g],
                    rhs=xbv()[:, :, c0 : c0 + cw],
                    start=True, stop=True,
                    perf_mode=DR,
                    skip_group_check=True,
                )
                r0 = c0 // 64
                inst = nc.vector.tensor_scalar(
                    out=v4pad[g][:, 66:1254].rearrange(
                        "p (r w) -> p r w", w=66)[:, r0 : r0 + cw // 64, 1:65],
                    in0=ps[:, 0:cw].rearrange("p (r w) -> p r w", w=64),
                    scalar1=IWS, scalar2=ball[:, BV + g : BV + g + 1],
                    op0=OP.mult, op1=OP.add,
                )
                v4_insts[g].append(inst)

            def dwv(g, t):
                base = DWG * g
                if t < 4:
                    sl = dwdiag[:, base + 256 * t : base + 256 * t + 256]
                    return sl.rearrange("p (k m) -> p k m", k=2)
                return dwdiag[:, base + 1024 : base + 1024 + 128]

            def dwconv(g):
                """Depthwise 3x3 on PE: diag-matmul taps into psum chunks."""
                for ch in range(3):
                    c0 = 352 * ch
                    ps = psp.tile([128, 512], f32, tag="acc", name="dw",
                                  bufs=2)
                    for t in range(4):
                        o0, o1 = DW_PAIRS[t]
                        rhs = v4pad[g][:, o0 + c0 : o0 + c0 + (o1 - o0) * 2
                                       : o1 - o0].unsqueeze(2)
                        rhs.ap[2] = [1, 352]
                        mm = nc.tensor.matmul(
                            ps[:, 0:352], lhsT=dwv(g, t), rhs=rhs,
                            start=(t == 0), stop=False,
                            perf_mode=DR,
                            skip_group_check=True,
                        )
                        for ci in v4_insts[g]:
                            add_dep_helper(mm.ins, ci.ins,
                                           reason="dwconv reads v4pad")
                    mm = nc.tensor.matmul(
                        ps[:, 0:352], lhsT=dwv(g, 4),
                        rhs=v4pad[g][:, DW_SINGLE + c0 : DW_SINGLE + c0 + 352],
                        start=False, stop=True,
                        skip_group_check=True,
                    )
                    for ci in v4_insts[g]:
                        add_dep_helper(mm.ins, ci.ins,
                                       reason="dwconv reads v4pad")
                    inst = nc.vector.tensor_scalar_mul(
                        out=pe_sb[g][:, c0 : c0 + 352], in0=ps[:, 0:352],
                        scalar1=IWS,
                    )
                dw_last[g] = inst

            # ---- attention ----
            def scores_mm(ncc, hg, half, j):
                s_ps = psp.tile([128, 1024], f32, tag="s", name="s", bufs=2)
                for hl in range(2):
                    h = 2 * half + hl
                    nc.tensor.matmul(
                        s_ps[:, 512 * hl : 512 * hl + 512],
                        lhsT=k_sb[hg][32 * h : 32 * h + 32,
                                      128 * j : 128 * j + 128],
                        rhs=q_sb[hg][32 * h : 32 * h + 32,
                                     512 * ncc : 512 * ncc + 512],
                        start=True, stop=True,
                        tile_position=(32 * h, 0),
                        skip_group_check=True,
                    )
                return s_ps

            def exp_mm(par, s_ps, pt2):
                nc.scalar.activation(
                    pt2[:, 1024 * par : 1024 * par + 1024].rearrange(
                        "q (h n) -> q h n", n=512
                    ),
                    s_ps[:].rearrange("q (h n) -> q h n", n=512),
                    AF.Exp, scale=SCALE,
                )

            def av_mm(hg, half, jp, pt2, av2):
                for m in range(2):
                    rhs = pt2[:].rearrange(
                        "q (pl h n) -> q pl h n", pl=2, n=512
                    )[:, :, m, :]
                    wbase = 128 * (4 * hg + 2 * m + half)
                    lhsT = vT2[jp][:].rearrange(
                        "q (pl w) -> q pl w", pl=2
                    )[:, :, wbase : wbase + 128]
                    nc.tensor.matmul(
                        av2[:],
                        lhsT=lhsT, rhs=rhs,
                        start=(jp == 0 and m == 0),
                        stop=(jp == 3 and m == 1),
                        perf_mode=DR,
                        skip_group_check=True,
                    )

            def attn_norm_a(ncc, hg, avpair):
                # avpair[0] rows: [av_h0|cs_h0|av_h1|cs_h1]
                # avpair[1] rows: [cs_h2|av_h2|cs_h3|av_h3]
                avs, rcp = [], []
                for i in range(2):
                    a = sp.tile([128, 512], f32, tag="avs", name="avs",
                                bufs=2)
                    nc.vector.tensor_copy(out=a[:], in_=avpair[i][:])
                    r = sp.tile([128, 512], f32, tag="rcp", name="rcp",
                                bufs=2)
                    nc.vector.reciprocal_approx_fast(out=r[:], in_=a[:])
                    avs.append(a)
                    rcp.append(r)
                avc = sp.tile([128, 512], f32, tag="avc", name="avc")
                rcpc = sp.tile([128, 512], f32, tag="rcpc", name="rcpc")
                # attn channel order per hg-block: local heads [0,2,1,3]
                gath = [
                    (avc, 0, avs[0], 0), (avc, 32, avs[1], 32),
                    (avc, 64, avs[0], 64), (avc, 96, avs[1], 96),
                    (rcpc, 0, rcp[0], 32), (rcpc, 32, rcp[1], 0),
                    (rcpc, 64, rcp[0], 96), (rcpc, 96, rcp[1], 64),
                ]
                for i, (dt_, do, st, so) in enumerate(gath):
                    qd = (nc.sync, nc.gpsimd)[i % 2]
                    qd.dma_start(out=dt_[do : do + 32],
                                 in_=st[so : so + 32])
                t1 = sp.tile([128, 512], bf16, tag=f"t1_{ncc}{hg}",
                             name="t1", bufs=1)
                nc.vector.tensor_mul(t1[:], avc[:], rcpc[:])
                return t1

            def attn_norm_b(ncc, hg, t1):
                inst = nc.vector.tensor_add(
                    attn2[:].rearrange("p (k n) -> p k n", k=2)[
                        :, hg, 512 * ncc : 512 * ncc + 512
                    ],
                    t1[:],
                    pe_sb[hg][:].rearrange("p (r w) -> p r w", w=66)[
                        :, 8 * ncc : 8 * ncc + 8, 1:65
                    ],
                )
                add_dep_helper(inst.ins, dw_last[hg].ins,
                               reason="norm_b reads pe")

            def mlp_block(ncc, as_thunks=False):
                thunks = []

                def emit(f):
                    if as_thunks:
                        thunks.append(f)
                    else:
                        f()

                use_act = ncc == 1  # ACT is idle in the tail
                s = slice(512 * ncc, 512 * ncc + 512)

                def proj_stage(g):
                    ps = psp.tile([128, 512], f32, tag="acc", name="proj",
                                  bufs=2)
                    nc.tensor.matmul(
                        ps[:],
                        lhsT=w2[:, W2_PROJ : W2_PROJ + 512].rearrange(
                            "p (k m) -> p k m", k=2
                        )[:, :, 128 * g : 128 * g + 128],
                        rhs=attn2[:].rearrange("p (k n) -> p k n", k=2)[
                            :, :, s
                        ],
                        start=True, stop=True,
                        perf_mode=DR,
                        skip_group_check=True,
                    )
                    nc.vector.scalar_tensor_tensor(
                        out=x1b2[:, NPX * g + 512 * ncc :
                                 NPX * g + 512 * ncc + 512],
                        in0=ps[:], scalar=IWS,
                        in1=xf1[g][:, s], op0=OP.mult, op1=OP.add,
                    )
                    nc.vector.scalar_tensor_tensor(
                        out=x1f[g][:, s], in0=ps[:], scalar=IWS,
                        in1=xf2[g][:, s], op0=OP.mult, op1=OP.add,
                    )

                for g in range(2):
                    emit(lambda g=g: proj_stage(g))

                def m1_stage(m):
                    mp = 128 if m < 2 else MLP - 256
                    ps = psp.tile([128, 512], f32, tag="acc", name="m1",
                                  bufs=2)
                    nc.tensor.matmul(
                        ps[:],
                        lhsT=w2[:, W2_M1 : W2_M1 + 768].rearrange(
                            "p (k m) -> p k m", k=2
                        )[:, :, 128 * m : 128 * m + 128],
                        rhs=x1b2[:].rearrange("p (k n) -> p k n", k=2)[
                            :, :, s
                        ],
                        start=True, stop=True,
                        perf_mode=DR,
                        skip_group_check=True,
                    )
                    th = sp.tile([128, 512], bf16, tag="tanh", name="th",
                                 bufs=3)
                    nc.scalar.activation(
                        th[:mp, :], ps[:mp, :], AF.Tanh,
                        bias=ball[:mp, BM1H + m : BM1H + m + 1],
                        scale=0.5 * IWS,
                    )
                    z = sp.tile([128, 512], bf16, tag="z", name="z", bufs=3)
                    if use_act:
                        nc.scalar.activation(
                            z[:mp, :], ps[:mp, :], AF.Identity,
                            bias=ball[:mp, BM1 + m : BM1 + m + 1],
                            scale=IWS,
                        )
                    else:
                        nc.vector.tensor_scalar(
                            out=z[:mp, :], in0=ps[:mp, :],
                            scalar1=IWS,
                            scalar2=ball[:mp, BM1 + m : BM1 + m + 1],
                            op0=OP.mult, op1=OP.add,
                        )
                    udst = (u01[:mp, NPX * m + 512 * ncc :
                                NPX * m + 512 * ncc + 512]
                            if m < 2 else u2[:mp, s])
                    nc.vector.scalar_tensor_tensor(
                        out=udst, in0=th[:mp, :], scalar=1.0,
                        in1=z[:mp, :], op0=OP.add, op1=OP.mult,
                    )

                for m in range(3):
                    emit(lambda m=m: m1_stage(m))

                def m2_stage(g):
                    ps = psp.tile([128, 512], f32, tag="acc", name="m2",
                                  bufs=2)
                    nc.tensor.matmul(
                        ps[:],
                        lhsT=w2[:, W2_M2 : W2_M2 + 512].rearrange(
                            "p (k m) -> p k m", k=2
                        )[:, :, 128 * g : 128 * g + 128],
                        rhs=u01[:].rearrange("p (k n) -> p k n", k=2)[
                            :, :, s
                        ],
                        start=True, stop=False,
                        perf_mode=DR,
                        skip_group_check=True,
                    )
                    kp = MLP - 256
                    nc.tensor.matmul(
                        ps[:],
                        lhsT=w2[:kp, W2_M2 + 512 + 128 * g :
                                W2_M2 + 512 + 128 * g + 128],
                        rhs=u2[:kp, s],
                        start=False, stop=True,
                        skip_group_check=True,
                    )
                    nc.vector.scalar_tensor_tensor(
                        out=out_sb[g][:, s], in0=ps[:], scalar=IWS,
                        in1=x1f[g][:, s], op0=OP.mult, op1=OP.add,
                    )
                    nc.sync.dma_start(
                        out=out_d[g, :, s], in_=out_sb[g][:, s]
                    )

                for g in range(2):
                    emit(lambda g=g: m2_stage(g))
                return thunks

            # ---- schedule ----
            qk_conv_chunk(0, 0, 0)
            qk_conv_chunk(1, 0, 0)
            fillers = [
                lambda: vt_conv(0),
                lambda: vt_conv(1),
                lambda: qk_conv_chunk(1, 0, 1),
                lambda: qk_conv_chunk(0, 0, 1),
            ]
            fillers += [lambda p=p: vt_conv(p) for p in range(2, 8)]
            fillers += [
                lambda: qk_conv_chunk(0, 1, 0),
                lambda: qk_conv_chunk(1, 1, 0),
                lambda: qk_conv_chunk(0, 1, 1),
                lambda: qk_conv_chunk(1, 1, 1),
            ]
            fillers += [
                lambda g=g, c0=c0, cw=cw: v4_chunk(g, c0, cw)
                for g in range(2)
                for c0, cw in ((0, 512), (512, 512), (1024, 128))
            ]
            sbs = [(ncc, hg, half) for ncc in range(2) for hg in range(2)
                   for half in range(2)]
            pending = [scores_mm(*sbs[0], 0), scores_mm(*sbs[0], 1)]
            t_norm = {}
            avpair = []
            fl = fillers
            for si, (ncc, hg, half) in enumerate(sbs):
                av2 = psp.tile([128, 512], f32, tag="av", name="av2",
                               bufs=2)
                pt2 = None
                for j in range(8):
                    jp, par = j // 2, j % 2
                    for _ in range(2):
                        if fl:
                            fl.pop(0)()
                    cur = pending.pop(0)
                    if par == 0:
                        pt2 = ptp.tile([128, 2048], f8, tag="pt",
                                       name="pt2")
                    exp_mm(par, cur, pt2)
                    if par == 1:
                        av_mm(hg, half, jp, pt2, av2)
                    nj = j + 2
                    if nj < 8:
                        pending.append(scores_mm(ncc, hg, half, nj))
                    elif si + 1 < len(sbs):
                        pending.append(scores_mm(*sbs[si + 1], nj - 8))
                avpair.append(av2)
                if half == 1:
                    p = si // 2
                    t_norm[p] = attn_norm_a(ncc, hg, avpair)
                    avpair = []
                    if p == 0:
                        while fl:  # all v4 chunks before dwconv
                            fl.pop(0)()
                        dwconv(0)
                        dwconv(1)
                    elif p == 1:
                        attn_norm_b(0, 0, t_norm[0])
                        attn_norm_b(0, 1, t_norm[1])
                        fl = mlp_block(0, as_thunks=True)
                    elif p == 2:
                        attn_norm_b(1, 0, t_norm[2])
            attn_norm_b(1, 1, t_norm[3])
            while fl:
                fl.pop(0)()
            mlp_block(1)

    nc.compile()
    return nc


def _get_graph():
    if "nc" not in _COMPILED:
        _COMPILED["nc"] = _build_graph()
    return _COMPILED["nc"]


def _prep_inputs(x, w_qk, s_qk, b_qk, w_v, s_v, b_v, w_pe, s_pe, b_pe,
                 w_proj, s_proj, b_proj, w_m1, s_m1, b_m1, w_m2, s_m2, b_m2):
    f32 = np.float32
    x = np.asarray(x, f32)
    w_qk = np.asarray(w_qk, f32) * np.asarray(s_qk, f32)[:, None]
    w_v_e = np.asarray(w_v, f32) * np.asarray(s_v, f32)[:, None]
    w_pe_e = np.asarray(w_pe, f32)[:, 0] * np.asarray(s_pe, f32)[:, None, None]
    w_proj_e = np.asarray(w_proj, f32) * np.asarray(s_proj, f32)[:, None]
    w_m1_e = np.asarray(w_m1, f32) * np.asarray(s_m1, f32)[:, None]
    w_m2_e = 0.5 * np.asarray(w_m2, f32) * np.asarray(s_m2, f32)[:, None]

    # w1 per k-tile: [qT 256 | kT 256 | vext 512]
    wqkT = np.concatenate([w_qk[:C].T, w_qk[C:].T], axis=1)  # [256, 512]
    wvT = w_v_e.T  # [256 ci, 256 co]
    vext = np.zeros((C, 512), f32)
    for h in range(8):
        hg, half, m = h // 4, (h % 4) // 2, h % 2
        vc = 256 * hg + 128 * m + 64 * half
        vext[:, vc : vc + 32] = wvT[:, 32 * h : 32 * h + 32]
    # attn channel order per hg-block: local heads [0,2,1,3]
    HPERM = [0, 2, 1, 3]
    cperm = np.concatenate([
        np.arange(32 * HPERM[i] + 128 * hg, 32 * HPERM[i] + 128 * hg + 32)
        for hg in range(2) for i in range(4)
    ])
    vdense = wvT[:, cperm]  # [256, 256] v weights in attn2 channel order
    w1full = np.concatenate([wqkT, vext, vdense], axis=1)  # [256, 1280]
    w1 = np.concatenate([w1full[:128], w1full[128:]], axis=1)  # [128, 2560]

    w2 = np.zeros((128, W2_TOT), f32)
    wprojT = w_proj_e.T[cperm]  # rows follow attn channel order
    w2[:, W2_PROJ : W2_PROJ + 256] = wprojT[:128]
    w2[:, W2_PROJ + 256 : W2_PROJ + 512] = wprojT[128:]
    wm1T = w_m1_e.T  # [256, 307]; 384-stride blocks, cols 307:384 zero
    w2[:, W2_M1 : W2_M1 + MLP] = wm1T[:128]
    w2[:, W2_M1 + 384 : W2_M1 + 384 + MLP] = wm1T[128:]
    wm2T = np.zeros((384, C), f32)
    wm2T[:MLP] = w_m2_e.T
    for kt in range(3):
        w2[:, W2_M2 + 256 * kt : W2_M2 + 256 * kt + 256] = wm2T[
            128 * kt : 128 * kt + 128
        ]

    b_qk = np.asarray(b_qk, f32)
    b_v = np.asarray(b_v, f32)
    b_pe = np.asarray(b_pe, f32)
    b_proj_eff = np.asarray(b_proj, f32) + w_proj_e @ b_pe
    b_m1_pad = np.zeros(384, f32)
    b_m1_pad[:MLP] = np.asarray(b_m1, f32)
    b_m2 = np.asarray(b_m2, f32)

    ball = np.zeros((128, 34), f32)
    ball[:, BQ : BQ + 2] = b_qk[:C].reshape(2, 128).T
    ball[:, BK : BK + 2] = b_qk[C:].reshape(2, 128).T
    ball[:, BV : BV + 2] = b_v.reshape(2, 128).T
    ball[:, BM1 : BM1 + 3] = b_m1_pad.reshape(3, 128).T
    ball[:, BM1H : BM1H + 3] = (0.5 * b_m1_pad).reshape(3, 128).T

    # bvrow: [WS*b_v(32) | WS(32)] per head in vext column order
    bvx = np.zeros(512, f32)
    for h in range(8):
        hg, half, m = h // 4, (h % 4) // 2, h % 2
        vc = 256 * hg + 128 * m + 64 * half
        bvx[vc : vc + 32] = WS * b_v[32 * h : 32 * h + 32]
        bvx[vc + 32 : vc + 64] = WS

    w_pe_flat = w_pe_e.reshape(C, 9)[cperm]  # rows in attn2 channel order
    dw = np.zeros((128, 2 * DWG), f32)
    for g in range(2):
        base = DWG * g
        ci = np.arange(128)
        for t, (o0, o1) in enumerate(DW_PAIRS):
            for pl, off in enumerate((o0, o1)):
                tap = DW_TAP_OF_OFF[off]
                dw[ci, base + 256 * t + 128 * pl + ci] = (
                    WS * w_pe_flat[128 * g + ci, tap]
                )
        tap = DW_TAP_OF_OFF[DW_SINGLE]
        dw[ci, base + 1024 + ci] = WS * w_pe_flat[128 * g + ci, tap]

    common = {
        "w1": np.clip(WS * w1, -240, 240).astype(F8),
        "w2": np.clip(WS * w2, -240, 240).astype(F8),
        "dw": np.clip(dw, -240, 240).astype(F8),
        "ball": ball,
        "bvrow": bvx.reshape(1, 512).astype(BF16),
    }

    in_maps = []
    for core in range(8):
        b, a = core // AREA, core % AREA
        xs = np.zeros((C, 18, W), f32)
        r0 = 16 * a - 1
        lo, hi = max(r0, 0), min(r0 + 18, H)
        xs[:, lo - r0 : lo - r0 + (hi - lo)] = x[b, :, lo:hi]
        m = dict(common)
        xbf = xs.reshape(C, NHALO)
        m["xb"] = np.clip(
            xbf.reshape(2, 128, NHALO).transpose(1, 0, 2), -240, 240
        ).astype(F8)
        xc = xs[:, 1:17].reshape(C, NPX)
        x1c = xc + b_proj_eff[:, None]
        x2c = x1c + b_m2[:, None]
        m["xf1"] = x1c.reshape(2, 128, NPX).astype(f32)
        m["xf2"] = x2c.reshape(2, 128, NPX).astype(f32)
        in_maps.append(m)
    return in_maps


def kernel(**inputs):
    from concourse.bass_utils import run_bass_kernel_spmd

    nc = _get_graph()
    in_maps = _prep_inputs(**inputs)
    res = run_bass_kernel_spmd(nc, in_maps, core_ids=list(range(8)))
    out = np.zeros((B, C, H, W), np.float32)
    for core in range(8):
        b, a = core // AREA, core % AREA
        o = np.asarray(res.results[core]["out"], np.float32).reshape(C, 16, W)
        out[b, :, 16 * a : 16 * a + 16, :] = o
    return out


# revision 18
# speedup vs baseline: 1.0167x; 1.0167x over previous
"""AreaAttentionBlock Trainium2 kernel (8 NeuronCores, data-parallel).

Problem: B=2, C=256, H=W=64, HEADS=8 (hd=32), AREA=4, MLP_DIM=307.
One (batch, area) group of 1024 pixels per core; the only cross-slab
dependency is the 1-row halo of the depthwise 3x3 conv, host-supplied.

Per-core pipeline:
  - All 1x1 convs run as fp8e4 DoubleRow matmuls (two K=128 k-tiles ride
    the DR pair dim). Weights host-scaled x64 into fp8's normal range;
    1/64 rides the psum->sbuf tensor_scalar ops; conv biases folded into
    host-prepared xf tiles.
  - Depthwise 3x3 conv on PE as diagonal-matrix fp8 DR matmuls over the
    zero-padded 18x66 v4 layout (tap pairs via overlapping strided APs).
  - Attention in 16 sub-blocks (ncc, hg, half): scores bf16 K=32;
    exp on ACT writes fp8 directly into j-pair plane tiles (pt2);
    AV+colsum fused: one fp8 DR M=128 matmul per (j-pair, head) with
    zero-padded window lhsT [vT_h|ones|0|0] so both heads of a half
    accumulate into ONE [128,512] psum tile holding [av|cs|av'|cs']
    (window order w=4hg+2m+half keeps vt scatter APs 3-free-dim).
    Norm: copy psum->sbuf, recip, 8 partition-shift gather DMAs build
    compact av and 1/cs, one mul; head order [0,2,1,3] absorbed into
    the host proj-weight permutation.
  - MLP silu via tanh (stays in exp ACT table set).
The HAM power governor throttles PE to 50% duty when PE streams exceed
~50% of wall; the DR design keeps attention-phase PE at ~48% so the exp
phase stays ACT-bound even when throttled.
"""

import numpy as np
import ml_dtypes

C = 256
HEADS = 8
HD = 32
AREA = 4
MLP = 307
B, H, W = 2, 64, 64
NPX = 1024          # pixels per slab (16 rows)
NHALO = 1152        # 18 rows with halo
SCALE = float(1.0 / np.sqrt(HD))
WS = 64.0           # host weight scale into fp8 normal range
IWS = float(1.0 / WS)

BF16 = ml_dtypes.bfloat16
F8 = ml_dtypes.float8_e4m3

# w1 free layout per k-tile: [qT 256 | kT 256 | vext 512 | vdense 256];
# vext has [v-cols(32) | zero-cols(32)] per head in window order (the
# zero block becomes the ones block via the bias matmul writing WS);
# vdense is the v weights densely in attn2's permuted head order, used
# by the v4 (image-layout) conv so pe matches attn2's channel order.
W1KT = 1280

# w2 packing offsets (wproj | wm1 | wm2 along free dim)
W2_PROJ = 0            # 2 x 256
W2_M1 = 512            # 2 x 384 (307 zero-padded to 16-aligned DR stride)
W2_M2 = 512 + 768      # 3 x 256
W2_TOT = W2_M2 + 768

# ball (f32 [128, 34]) column map
BQ, BK, BV = 0, 2, 4
BM1, BM1H = 10, 13

# dwdiag packing: per g, 4 DR pair tiles [128,2,128] + 1 single [128,128]
DW_PAIRS = [(65, 131), (66, 132), (67, 133), (197, 199)]
DW_SINGLE = 198
DW_TAP_OF_OFF = {65: 0, 66: 1, 67: 2, 131: 3, 132: 4, 133: 5,
                 197: 6, 198: 7, 199: 8}
DWG = 4 * 256 + 128    # 1152 cols per g

_COMPILED = {}


def _build_graph():
    import concourse.bacc as bacc
    import concourse.mybir as mybir
    import concourse.tile as tile
    from concourse.tile import add_dep_helper

    f32 = mybir.dt.float32
    bf16 = mybir.dt.bfloat16
    f8 = mybir.dt.float8e4
    DR = mybir.MatmulPerfMode.DoubleRow
    AF = mybir.ActivationFunctionType
    OP = mybir.AluOpType

    nc = bacc.Bacc(target_bir_lowering=False)

    xf1_d = nc.dram_tensor("xf1", [2, 128, NPX], f32, kind="ExternalInput")
    xf2_d = nc.dram_tensor("xf2", [2, 128, NPX], f32, kind="ExternalInput")
    xb_d = nc.dram_tensor("xb", [128, 2, NHALO], f8, kind="ExternalInput")
    w1_d = nc.dram_tensor("w1", [128, 2 * W1KT], f8, kind="ExternalInput")
    w2_d = nc.dram_tensor("w2", [128, W2_TOT], f8, kind="ExternalInput")
    dw_d = nc.dram_tensor("dw", [128, 2 * DWG], f8, kind="ExternalInput")
    ball_d = nc.dram_tensor("ball", [128, 34], f32, kind="ExternalInput")
    zeros_d = nc.dram_tensor("zeros", [128, 2048], f8, kind="ExternalInput")
    bvrow_d = nc.dram_tensor("bvrow", [1, 512], bf16, kind="ExternalInput")
    out_d = nc.dram_tensor("out", [2, 128, NPX], f32, kind="ExternalOutput")

    with tile.TileContext(nc) as tc:
        with (
            tc.sbuf_pool(name="weights", bufs=1) as wp,
            tc.sbuf_pool(name="acts", bufs=1) as ap,
            tc.sbuf_pool(name="pt_pool", bufs=3) as ptp,
            tc.sbuf_pool(name="small", bufs=2) as sp,
            tc.psum_pool(name="ps", bufs=1) as psp,
        ):
            # constants / ACT table preload
            onesrow = wp.tile([1, 128], bf16, name="onesrow")
            nc.vector.memset(onesrow[:], 1.0)
            warm = wp.tile([1, 16], f32, name="warm")
            # loads the exp ACT table set during the DMA phase
            nc.scalar.activation(warm[:], onesrow[:, 0:16], AF.Exp)

            # DMAs (ordered by first use)
            xb = ap.tile([128, 2 * NHALO], f8, name="xb")
            w1 = wp.tile([128, 2 * W1KT], f8, name="w1")
            ball = wp.tile([128, 34], f32, name="ball")

            def xbv():
                return xb[:].rearrange("p (k n) -> p k n", k=2)

            def w1v():
                return w1[:].rearrange("p (k m) -> p k m", k=2)

            nc.sync.dma_start(out=xbv()[:, :, 0:576],
                              in_=xb_d[:, :, 0:576])
            nc.gpsimd.dma_start(
                out=w1v()[:, :, 0:512],
                in_=w1_d[:].rearrange("p (k m) -> p k m", k=2)[:, :, 0:512])
            nc.sync.dma_start(out=ball[:], in_=ball_d[:])
            nc.gpsimd.dma_start(out=xbv()[:, :, 576:1152],
                                in_=xb_d[:, :, 576:1152])
            nc.sync.dma_start(
                out=w1v()[:, :, 512:1280],
                in_=w1_d[:].rearrange("p (k m) -> p k m", k=2)[:, :, 512:1280])
            bvrow = wp.tile([1, 512], bf16, name="bvrow")
            nc.gpsimd.dma_start(out=bvrow[:], in_=bvrow_d[:])
            w2 = wp.tile([128, W2_TOT], f8, name="w2")
            nc.gpsimd.dma_start(out=w2[:], in_=w2_d[:])
            dwdiag = wp.tile([128, 2 * DWG], f8, name="dwdiag")
            nc.gpsimd.dma_start(out=dwdiag[:], in_=dw_d[:])
            xf1 = [ap.tile([128, NPX], f32, name=f"xf1{k}") for k in range(2)]
            xf2 = [ap.tile([128, NPX], f32, name=f"xf2{k}") for k in range(2)]
            for k in range(2):
                nc.gpsimd.dma_start(out=xf1[k][:], in_=xf1_d[k])
                nc.sync.dma_start(out=xf2[k][:], in_=xf2_d[k])

            # persistent activation tiles
            q_sb = [ap.tile([128, NPX], bf16, name=f"q{g}") for g in range(2)]
            k_sb = [ap.tile([128, NPX], bf16, name=f"k{g}") for g in range(2)]
            # vT2[jp]: [pl(2) x 8 windows x 128] fp8; window w=4hg+2m+half:
            #   half0 m0: [vT|1|0|0]   half0 m1: [0|0|vT|1]
            #   half1 m0: [1|vT|0|0]   half1 m1: [0|0|1|vT]
            vT2 = [ap.tile([128, 2048], f8, name=f"vT2{jp}")
                   for jp in range(4)]
            for jp in range(4):
                (nc.sync, nc.gpsimd)[jp % 2].dma_start(
                    out=vT2[jp][:], in_=zeros_d[:])
            v4pad = [ap.tile([128, 1256], f8, name=f"v4p{g}")
                     for g in range(2)]
            pe_sb = [ap.tile([128, 1056], bf16, name=f"pe{g}")
                     for g in range(2)]
            attn2 = ap.tile([128, 2 * NPX], f8, name="attn2")
            x1f = [ap.tile([128, NPX], f32, name=f"x1f{g}") for g in range(2)]
            x1b2 = ap.tile([128, 2 * NPX], f8, name="x1b2")
            u01 = ap.tile([128, 2 * NPX], f8, name="u01")
            u2 = ap.tile([128, NPX], f8, name="u2")
            out_sb = [ap.tile([128, NPX], f32, name=f"osb{g}")
                      for g in range(2)]

            for g in range(2):
                (nc.sync, nc.gpsimd)[g].dma_start(
                    out=v4pad[g][:], in_=zeros_d[:, 0:1256])

            v4_insts = {0: [], 1: []}
            dw_last = {}

            # ---- conv building blocks ----
            def qk_conv_chunk(which, g, ncc):
                """One 512-px chunk of the q or k 1x1 conv (fp8 DR)."""
                dst = (q_sb, k_sb)[which]
                bias_col = (BQ, BK)[which] + g
                ps = psp.tile([128, 512], f32, tag="acc", name="qkc", bufs=2)
                mt = 256 * which + 128 * g
                nc.tensor.matmul(
                    ps[:],
                    lhsT=w1v()[:, :, mt : mt + 128],
                    rhs=xbv()[:, :, 64 + 512 * ncc : 64 + 512 * ncc + 512],
                    start=True, stop=True,
                    perf_mode=DR,
                    skip_group_check=True,
                )
                nc.vector.tensor_scalar(
                    out=dst[g][:, 512 * ncc : 512 * ncc + 512], in0=ps[:],
                    scalar1=IWS, scalar2=ball[:, bias_col : bias_col + 1],
                    op0=OP.mult, op1=OP.add,
                )

            def _win_ap(t, base, dims):
                """AP at column `base` with free dims `dims` ([stride,size]
                pairs) plus the trailing [1,32] block."""
                a = t[:, base : base + 32]
                for _ in dims:
                    a = a.unsqueeze(1)
                for i, ss in enumerate(dims):
                    a.ap[1 + i] = list(ss)
                return a

            def vt_conv(p):
                """V^T px-tile p -> vT2[p//2] plane p%2 window scatter."""
                ps = psp.tile([128, 512], f32, tag="acc", name="vtc", bufs=2)
                px0 = 64 + 128 * p
                nc.tensor.matmul(
                    ps[:],
                    lhsT=xbv()[:, :, px0 : px0 + 128],
                    rhs=w1v()[:, :, 512:1024],
                    start=True, stop=False,
                    perf_mode=DR,
                    skip_group_check=True,
                )
                nc.tensor.matmul(
                    ps[:], lhsT=onesrow[:], rhs=bvrow[:],
                    start=False, stop=True, skip_group_check=True,
                )
                jp, pl = p // 2, p % 2
                t = vT2[jp]
                base = 1024 * pl
                # head h=4hg+2half+m; psum v-col (host vext order):
                # 256hg+128m+64half; window w=4hg+2m+half at 128w with
                # inwin v at 64m+32half -> out = 512hg+320m+160half.
                # Both nest uniformly over k=2m+half (one 3-dim op).
                ov = _win_ap(t, base, [(512, 2), (160, 4)])
                iv = _win_ap(ps, 0, [(256, 2), (64, 4)])
                nc.vector.tensor_scalar_mul(out=ov, in0=iv, scalar1=IWS)
                # ones: out 512hg+320m+96half+32, in 256hg+128m+64half+32
                # -- split per hg to stay 3-dim
                for hg in range(2):
                    oo = _win_ap(t, base + 512 * hg + 32,
                                 [(320, 2), (96, 2)])
                    io = _win_ap(ps, 32 + 256 * hg, [(128, 2), (64, 2)])
                    nc.vector.tensor_scalar_mul(out=oo, in0=io, scalar1=IWS)

            def v4_chunk(g, c0, cw):
                """One chunk of the v 1x1 conv into the padded 18x66 layout."""
                ps = psp.tile([128, 512], f32, tag="acc", name="v4c", bufs=2)
                nc.tensor.matmul(
                    ps[:, 0:cw],
                    lhsT=w1v()[:, :, 1024 + 128 * g : 1152 + 128 * g],
                    rhs=xbv()[:, :, c0 : c0 + cw],
                    start=True, stop=True,
                    perf_mode=DR,
                    skip_group_check=True,
                )
                r0 = c0 // 64
                inst = nc.vector.tensor_scalar(
                    out=v4pad[g][:, 66:1254].rearrange(
                        "p (r w) -> p r w", w=66)[:, r0 : r0 + cw // 64, 1:65],
                    in0=ps[:, 0:cw].rearrange("p (r w) -> p r w", w=64),
                    scalar1=IWS, scalar2=ball[:, BV + g : BV + g + 1],
                    op0=OP.mult, op1=OP.add,
                )
                v4_insts[g].append(inst)

            def dwv(g, t):
                base = DWG * g
                if t < 4:
                    sl = dwdiag[:, base + 256 * t : base + 256 * t + 256]
                    return sl.rearrange("p (k m) -> p k m", k=2)
                return dwdiag[:, base + 1024 : base + 1024 + 128]

            def dwconv(g):
                """Depthwise 3x3 on PE: diag-matmul taps into psum chunks."""
                for ch in range(3):
                    c0 = 352 * ch
                    ps = psp.tile([128, 512], f32, tag="acc", name="dw",
                                  bufs=2)
                    for t in range(4):
                        o0, o1 = DW_PAIRS[t]
                        rhs = v4pad[g][:, o0 + c0 : o0 + c0 + (o1 - o0) * 2
                                       : o1 - o0].unsqueeze(2)
                        rhs.ap[2] = [1, 352]
                        mm = nc.tensor.matmul(
                            ps[:, 0:352], lhsT=dwv(g, t), rhs=rhs,
                            start=(t == 0), stop=False,
                            perf_mode=DR,
                            skip_group_check=True,
                        )
                        for ci in v4_insts[g]:
                            add_dep_helper(mm.ins, ci.ins,
                                           reason="dwconv reads v4pad")
                    mm = nc.tensor.matmul(
                        ps[:, 0:352], lhsT=dwv(g, 4),
                        rhs=v4pad[g][:, DW_SINGLE + c0 : DW_SINGLE + c0 + 352],
                        start=False, stop=True,
                        skip_group_check=True,
                    )
                    for ci in v4_insts[g]:
                        add_dep_helper(mm.ins, ci.ins,
                                       reason="dwconv reads v4pad")
                    inst = nc.vector.tensor_scalar_mul(
                        out=pe_sb[g][:, c0 : c0 + 352], in0=ps[:, 0:352],
                        scalar1=IWS,
                    )
                dw_last[g] = inst

            # ---- attention ----
            def scores_mm(ncc, hg, half, j):
                s_ps = psp.tile([128, 1024], f32, tag="s", name="s", bufs=2)
                for hl in range(2):
                    h = 2 * half + hl
                    nc.tensor.matmul(
                        s_ps[:, 512 * hl : 512 * hl + 512],
                        lhsT=k_sb[hg][32 * h : 32 * h + 32,
                                      128 * j : 128 * j + 128],
                        rhs=q_sb[hg][32 * h : 32 * h + 32,
                                     512 * ncc : 512 * ncc + 512],
                        start=True, stop=True,
                        tile_position=(32 * h, 0),
                        skip_group_check=True,
                    )
                return s_ps

            def exp_mm(par, s_ps, pt2):
                nc.scalar.activation(
                    pt2[:, 1024 * par : 1024 * par + 1024].rearrange(
                        "q (h n) -> q h n", n=512
                    ),
                    s_ps[:].rearrange("q (h n) -> q h n", n=512),
                    AF.Exp, scale=SCALE,
                )

            def av_mm(hg, half, jp, pt2, av2):
                for m in range(2):
                    rhs = pt2[:].rearrange(
                        "q (pl h n) -> q pl h n", pl=2, n=512
                    )[:, :, m, :]
                    wbase = 128 * (4 * hg + 2 * m + half)
                    lhsT = vT2[jp][:].rearrange(
                        "q (pl w) -> q pl w", pl=2
                    )[:, :, wbase : wbase + 128]
                    nc.tensor.matmul(
                        av2[:],
                        lhsT=lhsT, rhs=rhs,
                        start=(jp == 0 and m == 0),
                        stop=(jp == 3 and m == 1),
                        perf_mode=DR,
                        skip_group_check=True,
                    )

            def norm_copy(av2):
                # copy+recip right after the sub-block so av2 psum frees
                # before the next sub-block's first av matmul
                a = sp.tile([128, 512], f32, tag="avs", name="avs", bufs=3)
                nc.vector.tensor_copy(out=a[:], in_=av2[:])
                r = sp.tile([128, 512], f32, tag="rcp", name="rcp", bufs=3)
                nc.vector.reciprocal_approx_fast(out=r[:], in_=a[:])
                return a, r

            def attn_norm_a(ncc, hg, avpair):
                # avpair[0]=(avs,rcp) half0: rows [av_h0|cs_h0|av_h1|cs_h1]
                # avpair[1]=(avs,rcp) half1: rows [cs_h2|av_h2|cs_h3|av_h3]
                avs = [avpair[0][0], avpair[1][0]]
                rcp = [avpair[0][1], avpair[1][1]]
                avc = sp.tile([128, 512], f32, tag="avc", name="avc")
                rcpc = sp.tile([128, 512], f32, tag="rcpc", name="rcpc")
                # attn channel order per hg-block: local heads [0,2,1,3]
                gath = [
                    (avc, 0, avs[0], 0), (avc, 32, avs[1], 32),
                    (avc, 64, avs[0], 64), (avc, 96, avs[1], 96),
                    (rcpc, 0, rcp[0], 32), (rcpc, 32, rcp[1], 0),
                    (rcpc, 64, rcp[0], 96), (rcpc, 96, rcp[1], 64),
                ]
                for i, (dt_, do, st, so) in enumerate(gath):
                    qd = (nc.sync, nc.gpsimd)[i % 2]
                    qd.dma_start(out=dt_[do : do + 32],
                                 in_=st[so : so + 32])
                t1 = sp.tile([128, 512], bf16, tag=f"t1_{ncc}{hg}",
                             name="t1", bufs=1)
                nc.vector.tensor_mul(t1[:], avc[:], rcpc[:])
                return t1

            def attn_norm_b(ncc, hg, t1):
                inst = nc.vector.tensor_add(
                    attn2[:].rearrange("p (k n) -> p k n", k=2)[
                        :, hg, 512 * ncc : 512 * ncc + 512
                    ],
                    t1[:],
                    pe_sb[hg][:].rearrange("p (r w) -> p r w", w=66)[
                        :, 8 * ncc : 8 * ncc + 8, 1:65
                    ],
                )
                add_dep_helper(inst.ins, dw_last[hg].ins,
                               reason="norm_b reads pe")

            def mlp_block(ncc, as_thunks=False):
                thunks = []

                def emit(f):
                    if as_thunks:
                        thunks.append(f)
                    else:
                        f()

                use_act = ncc == 1  # ACT is idle in the tail
                s = slice(512 * ncc, 512 * ncc + 512)

                def proj_stage(g):
                    ps = psp.tile([128, 512], f32, tag="acc", name="proj",
                                  bufs=2)
                    nc.tensor.matmul(
                        ps[:],
                        lhsT=w2[:, W2_PROJ : W2_PROJ + 512].rearrange(
                            "p (k m) -> p k m", k=2
                        )[:, :, 128 * g : 128 * g + 128],
                        rhs=attn2[:].rearrange("p (k n) -> p k n", k=2)[
                            :, :, s
                        ],
                        start=True, stop=True,
                        perf_mode=DR,
                        skip_group_check=True,
                    )
                    nc.vector.scalar_tensor_tensor(
                        out=x1b2[:, NPX * g + 512 * ncc :
                                 NPX * g + 512 * ncc + 512],
                        in0=ps[:], scalar=IWS,
                        in1=xf1[g][:, s], op0=OP.mult, op1=OP.add,
                    )
                    nc.vector.scalar_tensor_tensor(
                        out=x1f[g][:, s], in0=ps[:], scalar=IWS,
                        in1=xf2[g][:, s], op0=OP.mult, op1=OP.add,
                    )

                for g in range(2):
                    emit(lambda g=g: proj_stage(g))

                def m1_stage(m):
                    mp = 128 if m < 2 else MLP - 256
                    ps = psp.tile([128, 512], f32, tag="acc", name="m1",
                                  bufs=2)
                    nc.tensor.matmul(
                        ps[:],
                        lhsT=w2[:, W2_M1 : W2_M1 + 768].rearrange(
                            "p (k m) -> p k m", k=2
                        )[:, :, 128 * m : 128 * m + 128],
                        rhs=x1b2[:].rearrange("p (k n) -> p k n", k=2)[
                            :, :, s
                        ],
                        start=True, stop=True,
                        perf_mode=DR,
                        skip_group_check=True,
                    )
                    th = sp.tile([128, 512], bf16, tag="tanh", name="th",
                                 bufs=3)
                    nc.scalar.activation(
                        th[:mp, :], ps[:mp, :], AF.Tanh,
                        bias=ball[:mp, BM1H + m : BM1H + m + 1],
                        scale=0.5 * IWS,
                    )
                    z = sp.tile([128, 512], bf16, tag="z", name="z", bufs=3)
                    if use_act:
                        nc.scalar.activation(
                            z[:mp, :], ps[:mp, :], AF.Identity,
                            bias=ball[:mp, BM1 + m : BM1 + m + 1],
                            scale=IWS,
                        )
                    else:
                        nc.vector.tensor_scalar(
                            out=z[:mp, :], in0=ps[:mp, :],
                            scalar1=IWS,
                            scalar2=ball[:mp, BM1 + m : BM1 + m + 1],
                            op0=OP.mult, op1=OP.add,
                        )
                    udst = (u01[:mp, NPX * m + 512 * ncc :
                                NPX * m + 512 * ncc + 512]
                            if m < 2 else u2[:mp, s])
                    nc.vector.scalar_tensor_tensor(
                        out=udst, in0=th[:mp, :], scalar=1.0,
                        in1=z[:mp, :], op0=OP.add, op1=OP.mult,
                    )

                for m in range(3):
                    emit(lambda m=m: m1_stage(m))

                def m2_stage(g):
                    ps = psp.tile([128, 512], f32, tag="acc", name="m2",
                                  bufs=2)
                    nc.tensor.matmul(
                        ps[:],
                        lhsT=w2[:, W2_M2 : W2_M2 + 512].rearrange(
                            "p (k m) -> p k m", k=2
                        )[:, :, 128 * g : 128 * g + 128],
                        rhs=u01[:].rearrange("p (k n) -> p k n", k=2)[
                            :, :, s
                        ],
                        start=True, stop=False,
                        perf_mode=DR,
                        skip_group_check=True,
                    )
                    kp = MLP - 256
                    nc.tensor.matmul(
                        ps[:],
                        lhsT=w2[:kp, W2_M2 + 512 + 128 * g :
                                W2_M2 + 512 + 128 * g + 128],
                        rhs=u2[:kp, s],
                        start=False, stop=True,
                        skip_group_check=True,
                    )
                    nc.vector.scalar_tensor_tensor(
                        out=out_sb[g][:, s], in0=ps[:], scalar=IWS,
                        in1=x1f[g][:, s], op0=OP.mult, op1=OP.add,
                    )
                    nc.sync.dma_start(
                        out=out_d[g, :, s], in_=out_sb[g][:, s]
                    )

                for g in range(2):
                    emit(lambda g=g: m2_stage(g))
                return thunks

            # ---- schedule ----
            qk_conv_chunk(0, 0, 0)
            qk_conv_chunk(1, 0, 0)
            fillers = [
                lambda: vt_conv(0),
                lambda: vt_conv(1),
                lambda: qk_conv_chunk(1, 0, 1),
                lambda: qk_conv_chunk(0, 0, 1),
            ]
            fillers += [lambda p=p: vt_conv(p) for p in range(2, 8)]
            fillers += [
                lambda: qk_conv_chunk(0, 1, 0),
                lambda: qk_conv_chunk(1, 1, 0),
                lambda: qk_conv_chunk(0, 1, 1),
                lambda: qk_conv_chunk(1, 1, 1),
            ]
            fillers += [
                lambda g=g, c0=c0, cw=cw: v4_chunk(g, c0, cw)
                for g in range(2)
                for c0, cw in ((0, 512), (512, 512), (1024, 128))
            ]
            sbs = [(ncc, hg, half) for ncc in range(2) for hg in range(2)
                   for half in range(2)]
            pending = [scores_mm(*sbs[0], 0), scores_mm(*sbs[0], 1)]
            t_norm = {}
            avpair = []
            fl = fillers
            for si, (ncc, hg, half) in enumerate(sbs):
                av2 = psp.tile([128, 512], f32, tag="av", name="av2",
                               bufs=2)
                pt2 = None
                for j in range(8):
                    jp, par = j // 2, j % 2
                    for _ in range(2):
                        if fl:
                            fl.pop(0)()
                    cur = pending.pop(0)
                    if par == 0:
                        pt2 = ptp.tile([128, 2048], f8, tag="pt",
                                       name="pt2")
                    exp_mm(par, cur, pt2)
                    if par == 1:
                        av_mm(hg, half, jp, pt2, av2)
                    nj = j + 2
                    if nj < 8:
                        pending.append(scores_mm(ncc, hg, half, nj))
                    elif si + 1 < len(sbs):
                        pending.append(scores_mm(*sbs[si + 1], nj - 8))
                avpair.append(norm_copy(av2))
                if half == 1:
                    p = si // 2
                    t_norm[p] = attn_norm_a(ncc, hg, avpair)
                    avpair = []
                    if p == 0:
                        while fl:  # all v4 chunks before dwconv
                            fl.pop(0)()
                        dwconv(0)
                        dwconv(1)
                    elif p == 1:
                        attn_norm_b(0, 0, t_norm[0])
                        attn_norm_b(0, 1, t_norm[1])
                        fl = mlp_block(0, as_thunks=True)
                    elif p == 2:
                        attn_norm_b(1, 0, t_norm[2])
            attn_norm_b(1, 1, t_norm[3])
            while fl:
                fl.pop(0)()
            mlp_block(1)

    nc.compile()
    return nc


def _get_graph():
    if "nc" not in _COMPILED:
        _COMPILED["nc"] = _build_graph()
    return _COMPILED["nc"]


def _prep_inputs(x, w_qk, s_qk, b_qk, w_v, s_v, b_v, w_pe, s_pe, b_pe,
                 w_proj, s_proj, b_proj, w_m1, s_m1, b_m1, w_m2, s_m2, b_m2):
    f32 = np.float32
    x = np.asarray(x, f32)
    w_qk = np.asarray(w_qk, f32) * np.asarray(s_qk, f32)[:, None]
    w_v_e = np.asarray(w_v, f32) * np.asarray(s_v, f32)[:, None]
    w_pe_e = np.asarray(w_pe, f32)[:, 0] * np.asarray(s_pe, f32)[:, None, None]
    w_proj_e = np.asarray(w_proj, f32) * np.asarray(s_proj, f32)[:, None]
    w_m1_e = np.asarray(w_m1, f32) * np.asarray(s_m1, f32)[:, None]
    w_m2_e = 0.5 * np.asarray(w_m2, f32) * np.asarray(s_m2, f32)[:, None]

    # w1 per k-tile: [qT 256 | kT 256 | vext 512]
    wqkT = np.concatenate([w_qk[:C].T, w_qk[C:].T], axis=1)  # [256, 512]
    wvT = w_v_e.T  # [256 ci, 256 co]
    vext = np.zeros((C, 512), f32)
    for h in range(8):
        hg, half, m = h // 4, (h % 4) // 2, h % 2
        vc = 256 * hg + 128 * m + 64 * half
        vext[:, vc : vc + 32] = wvT[:, 32 * h : 32 * h + 32]
    # attn channel order per hg-block: local heads [0,2,1,3]
    HPERM = [0, 2, 1, 3]
    cperm = np.concatenate([
        np.arange(32 * HPERM[i] + 128 * hg, 32 * HPERM[i] + 128 * hg + 32)
        for hg in range(2) for i in range(4)
    ])
    vdense = wvT[:, cperm]  # [256, 256] v weights in attn2 channel order
    w1full = np.concatenate([wqkT, vext, vdense], axis=1)  # [256, 1280]
    w1 = np.concatenate([w1full[:128], w1full[128:]], axis=1)  # [128, 2560]

    w2 = np.zeros((128, W2_TOT), f32)
    wprojT = w_proj_e.T[cperm]  # rows follow attn channel order
    w2[:, W2_PROJ : W2_PROJ + 256] = wprojT[:128]
    w2[:, W2_PROJ + 256 : W2_PROJ + 512] = wprojT[128:]
    wm1T = w_m1_e.T  # [256, 307]; 384-stride blocks, cols 307:384 zero
    w2[:, W2_M1 : W2_M1 + MLP] = wm1T[:128]
    w2[:, W2_M1 + 384 : W2_M1 + 384 + MLP] = wm1T[128:]
    wm2T = np.zeros((384, C), f32)
    wm2T[:MLP] = w_m2_e.T
    for kt in range(3):
        w2[:, W2_M2 + 256 * kt : W2_M2 + 256 * kt + 256] = wm2T[
            128 * kt : 128 * kt + 128
        ]

    b_qk = np.asarray(b_qk, f32)
    b_v = np.asarray(b_v, f32)
    b_pe = np.asarray(b_pe, f32)
    b_proj_eff = np.asarray(b_proj, f32) + w_proj_e @ b_pe
    b_m1_pad = np.zeros(384, f32)
    b_m1_pad[:MLP] = np.asarray(b_m1, f32)
    b_m2 = np.asarray(b_m2, f32)

    ball = np.zeros((128, 34), f32)
    ball[:, BQ : BQ + 2] = b_qk[:C].reshape(2, 128).T
    ball[:, BK : BK + 2] = b_qk[C:].reshape(2, 128).T
    ball[:, BV : BV + 2] = b_v.reshape(2, 128).T
    ball[:, BM1 : BM1 + 3] = b_m1_pad.reshape(3, 128).T
    ball[:, BM1H : BM1H + 3] = (0.5 * b_m1_pad).reshape(3, 128).T

    # bvrow: [WS*b_v(32) | WS(32)] per head in vext column order
    bvx = np.zeros(512, f32)
    for h in range(8):
        hg, half, m = h // 4, (h % 4) // 2, h % 2
        vc = 256 * hg + 128 * m + 64 * half
        bvx[vc : vc + 32] = WS * b_v[32 * h : 32 * h + 32]
        bvx[vc + 32 : vc + 64] = WS

    w_pe_flat = w_pe_e.reshape(C, 9)[cperm]  # rows in attn2 channel order
    dw = np.zeros((128, 2 * DWG), f32)
    for g in range(2):
        base = DWG * g
        ci = np.arange(128)
        for t, (o0, o1) in enumerate(DW_PAIRS):
            for pl, off in enumerate((o0, o1)):
                tap = DW_TAP_OF_OFF[off]
                dw[ci, base + 256 * t + 128 * pl + ci] = (
                    WS * w_pe_flat[128 * g + ci, tap]
                )
        tap = DW_TAP_OF_OFF[DW_SINGLE]
        dw[ci, base + 1024 + ci] = WS * w_pe_flat[128 * g + ci, tap]

    common = {
        "w1": np.clip(WS * w1, -240, 240).astype(F8),
        "w2": np.clip(WS * w2, -240, 240).astype(F8),
        "dw": np.clip(dw, -240, 240).astype(F8),
        "ball": ball,
        "zeros": np.zeros((128, 2048), F8),
        "bvrow": bvx.reshape(1, 512).astype(BF16),
    }

    in_maps = []
    for core in range(8):
        b, a = core // AREA, core % AREA
        xs = np.zeros((C, 18, W), f32)
        r0 = 16 * a - 1
        lo, hi = max(r0, 0), min(r0 + 18, H)
        xs[:, lo - r0 : lo - r0 + (hi - lo)] = x[b, :, lo:hi]
        m = dict(common)
        xbf = xs.reshape(C, NHALO)
        m["xb"] = np.clip(
            xbf.reshape(2, 128, NHALO).transpose(1, 0, 2), -240, 240
        ).astype(F8)
        xc = xs[:, 1:17].reshape(C, NPX)
        x1c = xc + b_proj_eff[:, None]
        x2c = x1c + b_m2[:, None]
        m["xf1"] = x1c.reshape(2, 128, NPX).astype(f32)
        m["xf2"] = x2c.reshape(2, 128, NPX).astype(f32)
        in_maps.append(m)
    return in_maps


def kernel(**inputs):
    from concourse.bass_utils import run_bass_kernel_spmd

    nc = _get_graph()
    in_maps = _prep_inputs(**inputs)
    res = run_bass_kernel_spmd(nc, in_maps, core_ids=list(range(8)))
    out = np.zeros((B, C, H, W), np.float32)
    for core in range(8):
        b, a = core // AREA, core % AREA
        o = np.asarray(res.results[core]["out"], np.float32).reshape(C, 16, W)
        out[b, :, 16 * a : 16 * a + 16, :] = o
    return out
